# revision 1
# baseline (speedup 1.0000x reference)
"""Trainium2 Bass kernel for top-2 MoE (nn_MoE_2113123910117).

Strategy (expert-parallel, per sharding hint):
  - Host: router logits -> softmax -> top-2 -> normalized combine weights;
    dispatch tokens to 8 expert shards (one expert per NeuronCore).
  - Device (per core): SwiGLU expert FFN over its gathered tokens,
    y = diag(scale) @ ((silu(x Wg^T) * (x Wu^T)) Wd^T), fp16 matmul
    operands with fp32 PSUM accumulation.
  - Host: scatter-add per-expert outputs back into the [B,T,D] output.

Self-contained: hardcodes all shapes from the problem spec.
"""

import os
import numpy as np

D = 1024
FF = 2048
E = 8
TOPK = 2
NCORES = 8
ND = D // 128    # 8 contraction chunks
NF = FF // 128   # 16 ff chunks
TT = 512         # token tile (moving-operand N per matmul)
MIN_CAP = 2176   # >= max expert load for the spec'd input, multiple of 128

# matmul operand dtype on device ("float16", "bfloat16")
MM_DTYPE = os.environ.get("MOE_MM_DTYPE", "float16")

# test-only knobs / results (harness never touches these)
LAST_RESULTS = None
_NC_CACHE = {}


def split_multi_waits(nc, mybir_mod):
    """This walrus build rejects any instruction carrying more than one
    sync wait ("Too many sync wait commands"). Hoist extra waits onto
    single-wait NOPs inserted just before the instruction on the same
    engine — semantically identical since engines execute in order."""
    n_split = 0
    for f in nc.m.functions:
        for blk in f.blocks:
            insts = blk.instructions
            newl = []
            changed = False
            for inst in insts:
                si = inst.sync_info
                if si is not None and len(si.on_wait) > 1:
                    waits = list(si.on_wait)
                    del si.on_wait[1:]
                    for j, w in enumerate(waits[1:]):
                        nop = mybir_mod.InstNoOp(
                            name=f"{inst.name}_w{j}",
                            engine=inst.engine,
                            ins=[],
                            outs=[],
                        )
                        nop.sync_info = mybir_mod.SyncInfo(on_wait=[w], on_update=[])
                        newl.append(nop)
                        n_split += 1
                    changed = True
                newl.append(inst)
            if changed:
                insts[:] = newl
    return n_split


def _token_tiles(cap):
    tiles = []
    off = 0
    while off < cap:
        tiles.append((off, min(TT, cap - off)))
        off += TT
    return tiles


def build_nc(cap, repeat=1):
    """Build the per-core Bass program: SwiGLU FFN for one expert over
    `cap` (padded) tokens. Same NEFF on all 8 cores (SPMD).

    repeat>1 wraps the whole body (including weight loads) in a hardware
    loop — used only for benchmarking (dispatch overhead amortization)."""
    import contextlib

    import concourse.bass as bass
    import concourse.mybir as mybir
    import concourse.tile as tile

    dt = mybir.dt
    f32 = dt.float32
    mmdt = getattr(dt, MM_DTYPE)
    AF = mybir.ActivationFunctionType
    NG = cap // 128  # token 128-groups

    nc = bass.Bass()
    xt = nc.dram_tensor("xt", [D, cap], mmdt, kind="ExternalInput")
    wg = nc.dram_tensor("wg", [D, FF], mmdt, kind="ExternalInput")
    wu = nc.dram_tensor("wu", [D, FF], mmdt, kind="ExternalInput")
    wd = nc.dram_tensor("wd", [FF, D], mmdt, kind="ExternalInput")
    sc = nc.dram_tensor("sc", [128, NG], f32, kind="ExternalInput")
    y = nc.dram_tensor("y", [cap, D], f32, kind="ExternalOutput")

    with tile.TileContext(nc) as tc:
        with (
            tc.tile_pool(name="wpool", bufs=1) as wpool,
            tc.tile_pool(name="xpool", bufs=2) as xpool,
            tc.tile_pool(name="hpool", bufs=2) as hpool,
            tc.tile_pool(name="gpool", bufs=3) as gpool,
            tc.tile_pool(name="ypool", bufs=3) as ypool,
            tc.tile_pool(name="pg", bufs=2, space="PSUM") as pgpool,
            tc.tile_pool(name="pu", bufs=2, space="PSUM") as pupool,
            tc.tile_pool(name="po", bufs=4, space="PSUM") as popool,
            (
                tc.For_i(0, repeat, 1, hint_engines=(mybir.EngineType.PE,))
                if repeat > 1
                else contextlib.nullcontext()
            ),
        ):
            # resident weights + combine scales
            wg_sb = []
            wu_sb = []
            wd_sb = []
            for d in range(ND):
                t = wpool.tile([128, FF], mmdt, tag=f"wg{d}")
                nc.sync.dma_start(t[:], wg[d * 128 : (d + 1) * 128, :])
                wg_sb.append(t)
            # first token tile's x loads go ahead of wu/wd so the gate
            # matmuls aren't queued behind the full 12.6MB weight stream
            # (TimelineSim showed a 34us PE stall there otherwise)
            tiles = _token_tiles(cap)
            off0, tt0 = tiles[0]
            xt0 = []
            for d in range(ND):
                t = xpool.tile([128, tt0], mmdt, tag=f"xt{d}")
                nc.sync.dma_start(t[:], xt[d * 128 : (d + 1) * 128, off0 : off0 + tt0])
                xt0.append(t)
            s_sb = wpool.tile([128, NG], f32, tag="s")
            nc.sync.dma_start(s_sb[:], sc[:])
            for d in range(ND):
                t = wpool.tile([128, FF], mmdt, tag=f"wu{d}")
                nc.sync.dma_start(t[:], wu[d * 128 : (d + 1) * 128, :])
                wu_sb.append(t)
            for f in range(NF):
                t = wpool.tile([128, D], mmdt, tag=f"wd{f}")
                nc.sync.dma_start(t[:], wd[f * 128 : (f + 1) * 128, :])
                wd_sb.append(t)

            for off, tt in tiles:
                # x^T tile: [d, tokens]
                if off == off0:
                    xt_t = xt0
                else:
                    xt_t = []
                    for d in range(ND):
                        t = xpool.tile([128, tt], mmdt, tag=f"xt{d}")
                        nc.sync.dma_start(
                            t[:], xt[d * 128 : (d + 1) * 128, off : off + tt]
                        )
                        xt_t.append(t)
                # gate/up + SwiGLU -> h^T [f, tokens]
                ht_t = []
                for f in range(NF):
                    pg = pgpool.tile([128, tt], f32, tag="pg")
                    pu = pupool.tile([128, tt], f32, tag="pu")
                    for d in range(ND):
                        nc.tensor.matmul(
                            pg[:],
                            wg_sb[d][:, f * 128 : (f + 1) * 128],
                            xt_t[d][:],
                            start=(d == 0),
                            stop=(d == ND - 1),
                        )
                    for d in range(ND):
                        nc.tensor.matmul(
                            pu[:],
                            wu_sb[d][:, f * 128 : (f + 1) * 128],
                            xt_t[d][:],
                            start=(d == 0),
                            stop=(d == ND - 1),
                        )
                    sg = gpool.tile([128, tt], mmdt, tag="sg")
                    nc.scalar.activation(sg[:], pg[:], AF.Silu)
                    ht = hpool.tile([128, tt], mmdt, tag=f"ht{f}")
                    nc.vector.tensor_mul(ht[:], sg[:], pu[:])
                    ht_t.append(ht)
                # down projection, scaled by combine weight per token
                for k in range(tt // 128):
                    g = off // 128 + k
                    po_h = []
                    for dh in range(2):
                        po = popool.tile([128, 512], f32, tag="po", name=f"po_{off}_{k}_{dh}")
                        po_h.append(po)
                    for f in range(NF):
                        lhs = ht_t[f][:, k * 128 : (k + 1) * 128]
                        for dh in range(2):
                            nc.tensor.matmul(
                                po_h[dh][:],
                                lhs,
                                wd_sb[f][:, dh * 512 : (dh + 1) * 512],
                                start=(f == 0),
                                stop=(f == NF - 1),
                            )
                    for dh in range(2):
                        yt = ypool.tile([128, 512], f32, tag="yt")
                        nc.scalar.activation(
                            yt[:], po_h[dh][:], AF.Copy, scale=s_sb[:, g : g + 1]
                        )
                        nc.sync.dma_start(
                            y[off + k * 128 : off + (k + 1) * 128,
                              dh * 512 : (dh + 1) * 512],
                            yt[:],
                        )
    split_multi_waits(nc, mybir)
    return nc


def _get_nc(cap):
    key = (cap, MM_DTYPE)
    if key not in _NC_CACHE:
        _NC_CACHE[key] = build_nc(cap)
    return _NC_CACHE[key]


def _route(xf, Wr):
    """fp32 softmax + top-2 + normalized combine weights, matching the
    jax reference (ties broken toward lower expert index)."""
    logits = xf @ Wr.astype(np.float32).T
    m = logits.max(-1, keepdims=True)
    ex = np.exp(logits - m)
    p = ex / ex.sum(-1, keepdims=True)
    top2 = np.argsort(-p, axis=-1, kind="stable")[:, :TOPK]
    n = xf.shape[0]
    p1 = p[np.arange(n), top2[:, 0]]
    p2 = p[np.arange(n), top2[:, 1]]
    denom = (p1 + p2) + np.float32(1e-8)
    return top2, p1 / denom, p2 / denom


def kernel(**inputs):
    global LAST_RESULTS
    from concourse.bass_utils import run_bass_kernel_spmd

    x = np.asarray(inputs["x"])
    Wr = np.asarray(inputs["Wr"])
    Wg = np.asarray(inputs["Wg"])
    Wu = np.asarray(inputs["Wu"])
    Wd = np.asarray(inputs["Wd"])
    B, T, _ = x.shape
    xf = x.reshape(-1, D).astype(np.float32, copy=False)
    n_tok = xf.shape[0]

    top2, s1, s2 = _route(xf, Wr)

    mmnp = np.dtype(np.float16 if MM_DTYPE == "float16" else np.float32)
    if MM_DTYPE == "bfloat16":
        import ml_dtypes

        mmnp = np.dtype(ml_dtypes.bfloat16)

    xf_mm = xf.astype(mmnp)

    idxs = []
    counts = []
    for e in range(E):
        idx = np.nonzero((top2[:, 0] == e) | (top2[:, 1] == e))[0]
        idxs.append(idx)
        counts.append(len(idx))
    cap = max(MIN_CAP, -(-max(counts) // 128) * 128)

    in_maps = []
    for e in range(E):
        idx = idxs[e]
        n_e = len(idx)
        xt = np.zeros((D, cap), dtype=mmnp)
        xt[:, :n_e] = xf_mm[idx].T
        sc = np.zeros(cap, dtype=np.float32)
        sc[:n_e] = np.where(top2[idx, 0] == e, s1[idx], s2[idx])
        sc2d = np.ascontiguousarray(sc.reshape(cap // 128, 128).T)
        in_maps.append(
            {
                "xt": xt,
                "wg": np.ascontiguousarray(Wg[e].T).astype(mmnp),
                "wu": np.ascontiguousarray(Wu[e].T).astype(mmnp),
                "wd": np.ascontiguousarray(Wd[e].T).astype(mmnp),
                "sc": sc2d,
            }
        )

    nc = _get_nc(cap)
    res = run_bass_kernel_spmd(nc, in_maps, list(range(NCORES)))
    LAST_RESULTS = res

    out = np.zeros((n_tok, D), dtype=np.float32)
    for e in range(E):
        idx = idxs[e]
        out[idx] += res.results[e]["y"][: len(idx)]
    return out.reshape(B, T, D).astype(x.dtype, copy=False)



# revision 14
# speedup vs baseline: 1.8356x; 1.8356x over previous
"""Trainium2 Bass kernel for top-2 MoE (nn_MoE_2113123910117).

Strategy (expert-parallel, per sharding hint):
  - Host: router logits -> softmax -> top-2 -> normalized combine weights;
    dispatch tokens to 8 expert shards (one expert per NeuronCore).
  - Device (per core): SwiGLU expert FFN over its gathered tokens,
    y = diag(scale) @ ((silu(x Wg^T) * (x Wu^T)) Wd^T), fp16 matmul
    operands with fp32 PSUM accumulation.
  - Host: scatter-add per-expert outputs back into the [B,T,D] output.

Self-contained: hardcodes all shapes from the problem spec.
"""

import os
import numpy as np

D = 1024
FF = 2048
E = 8
TOPK = 2
NCORES = 8
ND = D // 128    # 8 contraction chunks
NF = FF // 128   # 16 ff chunks
TT = 512         # token tile (moving-operand N per matmul)
MIN_CAP = 2176   # >= max expert load for the spec'd input, multiple of 128

# matmul operand dtype on device ("float16", "bfloat16")
MM_DTYPE = os.environ.get("MOE_MM_DTYPE", "float16")

# test-only knobs / results (harness never touches these)
LAST_RESULTS = None
_NC_CACHE = {}


def split_multi_waits(nc, mybir_mod):
    """This walrus build rejects any instruction carrying more than one
    sync wait ("Too many sync wait commands"). Hoist extra waits onto
    single-wait NOPs inserted just before the instruction on the same
    engine — semantically identical since engines execute in order."""
    n_split = 0
    for f in nc.m.functions:
        for blk in f.blocks:
            insts = blk.instructions
            newl = []
            changed = False
            for inst in insts:
                si = inst.sync_info
                if si is not None and len(si.on_wait) > 1:
                    waits = list(si.on_wait)
                    del si.on_wait[1:]
                    for j, w in enumerate(waits[1:]):
                        nop = mybir_mod.InstNoOp(
                            name=f"{inst.name}_w{j}",
                            engine=inst.engine,
                            ins=[],
                            outs=[],
                        )
                        nop.sync_info = mybir_mod.SyncInfo(on_wait=[w], on_update=[])
                        newl.append(nop)
                        n_split += 1
                    changed = True
                newl.append(inst)
            if changed:
                insts[:] = newl
    return n_split


def _token_tiles(cap):
    tiles = []
    off = 0
    while off < cap:
        tiles.append((off, min(TT, cap - off)))
        off += TT
    return tiles


def build_nc(cap, repeat=1):
    """Build the per-core Bass program: SwiGLU FFN for one expert over
    `cap` (padded) tokens. Same NEFF on all 8 cores (SPMD).

    repeat>1 wraps the whole body (including weight loads) in a hardware
    loop — used only for benchmarking (dispatch overhead amortization)."""
    import contextlib

    import concourse.bass as bass
    import concourse.mybir as mybir
    import concourse.tile as tile

    dt = mybir.dt
    f32 = dt.float32
    mmdt = getattr(dt, MM_DTYPE)
    AF = mybir.ActivationFunctionType
    NG = cap // 128  # token 128-groups

    nc = bass.Bass()
    xt = nc.dram_tensor("xt", [D, cap], mmdt, kind="ExternalInput")
    wg = nc.dram_tensor("wg", [D, FF], mmdt, kind="ExternalInput")
    wu = nc.dram_tensor("wu", [D, FF], mmdt, kind="ExternalInput")
    wd = nc.dram_tensor("wd", [FF, D], mmdt, kind="ExternalInput")
    sc = nc.dram_tensor("sc", [128, NG], f32, kind="ExternalInput")
    y = nc.dram_tensor("y", [cap, D], mmdt, kind="ExternalOutput")

    with tile.TileContext(nc) as tc:
        with (
            tc.tile_pool(name="wpool", bufs=1) as wpool,
            tc.tile_pool(name="xpool", bufs=2) as xpool,
            tc.tile_pool(name="hpool", bufs=2) as hpool,
            tc.tile_pool(name="gpool", bufs=3) as gpool,
            tc.tile_pool(name="ypool", bufs=3) as ypool,
            tc.tile_pool(name="pg", bufs=2, space="PSUM") as pgpool,
            tc.tile_pool(name="pu", bufs=2, space="PSUM") as pupool,
            tc.tile_pool(name="po", bufs=4, space="PSUM") as popool,
            (
                tc.For_i(0, repeat, 1, hint_engines=(mybir.EngineType.PE,))
                if repeat > 1
                else contextlib.nullcontext()
            ),
        ):
            # x for the first token tile loads ahead of everything so the
            # first gate matmuls only wait on it + the first wg/wu column
            # round. wg/wu stream in interleaved 512-col rounds (the PE
            # consumes them f-tile by f-tile, so full-tensor loads would
            # serialize ~13MB before the first matmul); wd follows (first
            # needed only after the whole first tile's gate/up).
            tiles = _token_tiles(cap)
            off0, tt0 = tiles[0]
            xt0 = []
            for d in range(ND):
                t = xpool.tile([128, tt0], mmdt, tag=f"xt{d}")
                nc.sync.dma_start(t[:], xt[d * 128 : (d + 1) * 128, off0 : off0 + tt0])
                xt0.append(t)
            s_sb = wpool.tile([128, NG], f32, tag="s")
            nc.sync.dma_start(s_sb[:], sc[:])
            wg_sb = []
            wu_sb = []
            wd_sb = []
            for d in range(ND):
                wg_sb.append(
                    wpool.tile([128, FF], mmdt, tag=f"wg{d}", name=f"wg_sb{d}")
                )
                wu_sb.append(
                    wpool.tile([128, FF], mmdt, tag=f"wu{d}", name=f"wu_sb{d}")
                )
            RC = 512  # column-round width
            for c0 in range(0, FF, RC):
                for d in range(ND):
                    nc.sync.dma_start(
                        wg_sb[d][:, c0 : c0 + RC],
                        wg[d * 128 : (d + 1) * 128, c0 : c0 + RC],
                    )
                for d in range(ND):
                    nc.sync.dma_start(
                        wu_sb[d][:, c0 : c0 + RC],
                        wu[d * 128 : (d + 1) * 128, c0 : c0 + RC],
                    )
            for f in range(NF):
                t = wpool.tile([128, D], mmdt, tag=f"wd{f}")
                nc.sync.dma_start(t[:], wd[f * 128 : (f + 1) * 128, :])
                wd_sb.append(t)

            for off, tt in tiles:
                # x^T tile: [d, tokens]
                if off == off0:
                    xt_t = xt0
                else:
                    xt_t = []
                    for d in range(ND):
                        t = xpool.tile([128, tt], mmdt, tag=f"xt{d}")
                        nc.sync.dma_start(
                            t[:], xt[d * 128 : (d + 1) * 128, off : off + tt]
                        )
                        xt_t.append(t)
                # gate/up + SwiGLU -> h^T [f, tokens]
                ht_t = []
                for f in range(NF):
                    pg = pgpool.tile([128, tt], f32, tag="pg")
                    pu = pupool.tile([128, tt], f32, tag="pu")
                    for d in range(ND):
                        nc.tensor.matmul(
                            pg[:],
                            wg_sb[d][:, f * 128 : (f + 1) * 128],
                            xt_t[d][:],
                            start=(d == 0),
                            stop=(d == ND - 1),
                        )
                    for d in range(ND):
                        nc.tensor.matmul(
                            pu[:],
                            wu_sb[d][:, f * 128 : (f + 1) * 128],
                            xt_t[d][:],
                            start=(d == 0),
                            stop=(d == ND - 1),
                        )
                    sg = gpool.tile([128, tt], mmdt, tag="sg")
                    nc.scalar.activation(sg[:], pg[:], AF.Silu)
                    ht = hpool.tile([128, tt], mmdt, tag=f"ht{f}")
                    nc.vector.tensor_mul(ht[:], sg[:], pu[:])
                    ht_t.append(ht)
                # down projection, scaled by combine weight per token
                for k in range(tt // 128):
                    g = off // 128 + k
                    po_h = []
                    for dh in range(2):
                        po = popool.tile([128, 512], f32, tag="po", name=f"po_{off}_{k}_{dh}")
                        po_h.append(po)
                    for f in range(NF):
                        lhs = ht_t[f][:, k * 128 : (k + 1) * 128]
                        for dh in range(2):
                            nc.tensor.matmul(
                                po_h[dh][:],
                                lhs,
                                wd_sb[f][:, dh * 512 : (dh + 1) * 512],
                                start=(f == 0),
                                stop=(f == NF - 1),
                            )
                    for dh in range(2):
                        yt = ypool.tile([128, 512], mmdt, tag="yt")
                        nc.scalar.activation(
                            yt[:], po_h[dh][:], AF.Copy, scale=s_sb[:, g : g + 1]
                        )
                        nc.sync.dma_start(
                            y[off + k * 128 : off + (k + 1) * 128,
                              dh * 512 : (dh + 1) * 512],
                            yt[:],
                        )
    split_multi_waits(nc, mybir)
    return nc


def _get_nc(cap):
    key = (cap, MM_DTYPE)
    if key not in _NC_CACHE:
        _NC_CACHE[key] = build_nc(cap)
    return _NC_CACHE[key]


def _route(xf, Wr):
    """fp32 softmax + top-2 + normalized combine weights, matching the
    jax reference (ties broken toward lower expert index)."""
    logits = xf @ Wr.astype(np.float32).T
    m = logits.max(-1, keepdims=True)
    ex = np.exp(logits - m)
    p = ex / ex.sum(-1, keepdims=True)
    top2 = np.argsort(-p, axis=-1, kind="stable")[:, :TOPK]
    n = xf.shape[0]
    p1 = p[np.arange(n), top2[:, 0]]
    p2 = p[np.arange(n), top2[:, 1]]
    denom = (p1 + p2) + np.float32(1e-8)
    return top2, p1 / denom, p2 / denom


def kernel(**inputs):
    global LAST_RESULTS
    from concourse.bass_utils import run_bass_kernel_spmd

    x = np.asarray(inputs["x"])
    Wr = np.asarray(inputs["Wr"])
    Wg = np.asarray(inputs["Wg"])
    Wu = np.asarray(inputs["Wu"])
    Wd = np.asarray(inputs["Wd"])
    B, T, _ = x.shape
    xf = x.reshape(-1, D).astype(np.float32, copy=False)
    n_tok = xf.shape[0]

    top2, s1, s2 = _route(xf, Wr)

    mmnp = np.dtype(np.float16 if MM_DTYPE == "float16" else np.float32)
    if MM_DTYPE == "bfloat16":
        import ml_dtypes

        mmnp = np.dtype(ml_dtypes.bfloat16)

    xf_mm = xf.astype(mmnp)

    idxs = []
    counts = []
    for e in range(E):
        idx = np.nonzero((top2[:, 0] == e) | (top2[:, 1] == e))[0]
        idxs.append(idx)
        counts.append(len(idx))
    cap = max(MIN_CAP, -(-max(counts) // 128) * 128)

    in_maps = []
    for e in range(E):
        idx = idxs[e]
        n_e = len(idx)
        xt = np.zeros((D, cap), dtype=mmnp)
        xt[:, :n_e] = xf_mm[idx].T
        sc = np.zeros(cap, dtype=np.float32)
        sc[:n_e] = np.where(top2[idx, 0] == e, s1[idx], s2[idx])
        sc2d = np.ascontiguousarray(sc.reshape(cap // 128, 128).T)
        in_maps.append(
            {
                "xt": xt,
                "wg": np.ascontiguousarray(Wg[e].T).astype(mmnp),
                "wu": np.ascontiguousarray(Wu[e].T).astype(mmnp),
                "wd": np.ascontiguousarray(Wd[e].T).astype(mmnp),
                "sc": sc2d,
            }
        )

    nc = _get_nc(cap)
    res = run_bass_kernel_spmd(nc, in_maps, list(range(NCORES)))
    LAST_RESULTS = res

    out = np.zeros((n_tok, D), dtype=np.float32)
    for e in range(E):
        idx = idxs[e]
        out[idx] += res.results[e]["y"][: len(idx)]
    return out.reshape(B, T, D).astype(x.dtype, copy=False)



# revision 18
# speedup vs baseline: 2.1851x; 1.1904x over previous
"""Trainium2 Bass kernel for top-2 MoE (nn_MoE_2113123910117).

Strategy (expert-parallel, per sharding hint):
  - Host: router logits -> softmax -> top-2 -> normalized combine weights;
    dispatch tokens to 8 expert shards (one expert per NeuronCore).
  - Device (per core): SwiGLU expert FFN over its gathered tokens,
    y = diag(scale) @ ((silu(x Wg^T) * (x Wu^T)) Wd^T), fp16 matmul
    operands with fp32 PSUM accumulation.
  - Host: scatter-add per-expert outputs back into the [B,T,D] output.

Self-contained: hardcodes all shapes from the problem spec.
"""

import os
import numpy as np

D = 1024
FF = 2048
E = 8
TOPK = 2
NCORES = 8
ND = D // 128    # 8 contraction chunks
NF = FF // 128   # 16 ff chunks
TT = 512         # token tile (moving-operand N per matmul)
MIN_CAP = 2176   # >= max expert load for the spec'd input, multiple of 128

# matmul operand dtype on device ("float16", "bfloat16")
MM_DTYPE = os.environ.get("MOE_MM_DTYPE", "float16")

# test-only knobs / results (harness never touches these)
LAST_RESULTS = None
_NC_CACHE = {}


def split_multi_waits(nc, mybir_mod):
    """This walrus build rejects any instruction carrying more than one
    sync wait ("Too many sync wait commands"). Hoist extra waits onto
    single-wait NOPs inserted just before the instruction on the same
    engine — semantically identical since engines execute in order."""
    n_split = 0
    for f in nc.m.functions:
        for blk in f.blocks:
            insts = blk.instructions
            newl = []
            changed = False
            for inst in insts:
                si = inst.sync_info
                if si is not None and len(si.on_wait) > 1:
                    waits = list(si.on_wait)
                    del si.on_wait[1:]
                    for j, w in enumerate(waits[1:]):
                        nop = mybir_mod.InstNoOp(
                            name=f"{inst.name}_w{j}",
                            engine=inst.engine,
                            ins=[],
                            outs=[],
                        )
                        nop.sync_info = mybir_mod.SyncInfo(on_wait=[w], on_update=[])
                        newl.append(nop)
                        n_split += 1
                    changed = True
                newl.append(inst)
            if changed:
                insts[:] = newl
    return n_split


def _token_tiles(cap):
    tiles = []
    off = 0
    while off < cap:
        tiles.append((off, min(TT, cap - off)))
        off += TT
    return tiles


def build_nc(cap, repeat=1):
    """Build the per-core Bass program: SwiGLU FFN for one expert over
    `cap` (padded) tokens. Same NEFF on all 8 cores (SPMD).

    repeat>1 wraps the whole body (including weight loads) in a hardware
    loop — used only for benchmarking (dispatch overhead amortization)."""
    import contextlib

    import concourse.bass as bass
    import concourse.mybir as mybir
    import concourse.tile as tile

    dt = mybir.dt
    f32 = dt.float32
    mmdt = getattr(dt, MM_DTYPE)
    AF = mybir.ActivationFunctionType
    NG = cap // 128  # token 128-groups

    nc = bass.Bass()
    # 3D dram layouts so a whole [128, chunks, cols] SBUF tile fills in
    # one descriptor (row-major identical to the natural 2D layouts) —
    # many small per-d-chunk DMAs serialize on the DGE queue
    xt = nc.dram_tensor("xt", [ND, 128, cap], mmdt, kind="ExternalInput")
    wg = nc.dram_tensor("wg", [ND, 128, FF], mmdt, kind="ExternalInput")
    wu = nc.dram_tensor("wu", [ND, 128, FF], mmdt, kind="ExternalInput")
    wd = nc.dram_tensor("wd", [NF, 128, D], mmdt, kind="ExternalInput")
    sc = nc.dram_tensor("sc", [128, NG], f32, kind="ExternalInput")
    y = nc.dram_tensor("y", [NG, 128, D], mmdt, kind="ExternalOutput")

    with tile.TileContext(nc) as tc:
        with (
            tc.tile_pool(name="wpool", bufs=1) as wpool,
            tc.tile_pool(name="xpool", bufs=3) as xpool,
            tc.tile_pool(name="hpool", bufs=2) as hpool,
            tc.tile_pool(name="gpool", bufs=3) as gpool,
            tc.tile_pool(name="ypool", bufs=2) as ypool,
            tc.tile_pool(name="pg", bufs=2, space="PSUM") as pgpool,
            tc.tile_pool(name="pu", bufs=2, space="PSUM") as pupool,
            tc.tile_pool(name="po", bufs=4, space="PSUM") as popool,
            (
                tc.For_i(0, repeat, 1, hint_engines=(mybir.EngineType.PE,))
                if repeat > 1
                else contextlib.nullcontext()
            ),
        ):
            # x for the first token tile loads ahead of everything so the
            # first gate matmuls only wait on it + the first wg/wu column
            # round. wg/wu stream in interleaved 512-col rounds (the PE
            # consumes them f-tile by f-tile, so full-tensor loads would
            # serialize ~13MB before the first matmul); wd follows (first
            # needed only after the whole first tile's gate/up).
            tiles = _token_tiles(cap)
            off0, tt0 = tiles[0]
            xt0 = xpool.tile([128, ND, tt0], mmdt, tag="xt")
            nc.sync.dma_start(
                xt0[:], xt[:, :, off0 : off0 + tt0].rearrange("d p t -> p d t")
            )
            s_sb = wpool.tile([128, NG], f32, tag="s")
            nc.sync.dma_start(s_sb[:], sc[:])
            wg_sb = wpool.tile([128, ND, FF], mmdt, tag="wg")
            wu_sb = wpool.tile([128, ND, FF], mmdt, tag="wu")
            wd_sb = wpool.tile([128, NF, D], mmdt, tag="wd")
            RC = 512  # column-round width
            for c0 in range(0, FF, RC):
                nc.sync.dma_start(
                    wg_sb[:, :, c0 : c0 + RC],
                    wg[:, :, c0 : c0 + RC].rearrange("d p c -> p d c"),
                )
                nc.sync.dma_start(
                    wu_sb[:, :, c0 : c0 + RC],
                    wu[:, :, c0 : c0 + RC].rearrange("d p c -> p d c"),
                )
            for f0 in range(0, NF, 8):
                nc.sync.dma_start(
                    wd_sb[:, f0 : f0 + 8, :],
                    wd[f0 : f0 + 8, :, :].rearrange("f p c -> p f c"),
                )

            for off, tt in tiles:
                # x^T tile: [d, tokens]
                if off == off0:
                    xt_t = xt0
                else:
                    xt_t = xpool.tile([128, ND, tt], mmdt, tag="xt")
                    nc.sync.dma_start(
                        xt_t[:], xt[:, :, off : off + tt].rearrange("d p t -> p d t")
                    )
                # gate/up + SwiGLU -> h^T [f, tokens]
                ht_t = []
                for f in range(NF):
                    pg = pgpool.tile([128, tt], f32, tag="pg")
                    pu = pupool.tile([128, tt], f32, tag="pu")
                    for d in range(ND):
                        nc.tensor.matmul(
                            pg[:],
                            wg_sb[:, d, f * 128 : (f + 1) * 128],
                            xt_t[:, d, :],
                            start=(d == 0),
                            stop=(d == ND - 1),
                        )
                    for d in range(ND):
                        nc.tensor.matmul(
                            pu[:],
                            wu_sb[:, d, f * 128 : (f + 1) * 128],
                            xt_t[:, d, :],
                            start=(d == 0),
                            stop=(d == ND - 1),
                        )
                    sg = gpool.tile([128, tt], mmdt, tag="sg")
                    nc.scalar.activation(sg[:], pg[:], AF.Silu)
                    ht = hpool.tile([128, tt], mmdt, tag=f"ht{f}")
                    nc.vector.tensor_mul(ht[:], sg[:], pu[:])
                    ht_t.append(ht)
                # down projection, scaled by combine weight per token
                nk = tt // 128
                yt = ypool.tile([128, nk, D], mmdt, tag="yt", name=f"yt_{off}")
                for k in range(nk):
                    g = off // 128 + k
                    po_h = []
                    for dh in range(2):
                        po = popool.tile([128, 512], f32, tag="po", name=f"po_{off}_{k}_{dh}")
                        po_h.append(po)
                    for f in range(NF):
                        lhs = ht_t[f][:, k * 128 : (k + 1) * 128]
                        for dh in range(2):
                            nc.tensor.matmul(
                                po_h[dh][:],
                                lhs,
                                wd_sb[:, f, dh * 512 : (dh + 1) * 512],
                                start=(f == 0),
                                stop=(f == NF - 1),
                            )
                    nc.scalar.activation(
                        yt[:, k, 0:512],
                        po_h[0][:],
                        AF.Copy,
                        scale=s_sb[:, g : g + 1],
                    )
                    nc.vector.tensor_scalar_mul(
                        yt[:, k, 512:1024], po_h[1][:], s_sb[:, g : g + 1]
                    )
                g0 = off // 128
                nc.sync.dma_start(
                    y[g0 : g0 + nk, :, :].rearrange("g p c -> p g c"), yt[:]
                )
    split_multi_waits(nc, mybir)
    return nc


def _get_nc(cap):
    key = (cap, MM_DTYPE)
    if key not in _NC_CACHE:
        _NC_CACHE[key] = build_nc(cap)
    return _NC_CACHE[key]


def _route(xf, Wr):
    """fp32 softmax + top-2 + normalized combine weights, matching the
    jax reference (ties broken toward lower expert index)."""
    logits = xf @ Wr.astype(np.float32).T
    m = logits.max(-1, keepdims=True)
    ex = np.exp(logits - m)
    p = ex / ex.sum(-1, keepdims=True)
    top2 = np.argsort(-p, axis=-1, kind="stable")[:, :TOPK]
    n = xf.shape[0]
    p1 = p[np.arange(n), top2[:, 0]]
    p2 = p[np.arange(n), top2[:, 1]]
    denom = (p1 + p2) + np.float32(1e-8)
    return top2, p1 / denom, p2 / denom


def prepare(inputs):
    """Host dispatch: returns (cap, idxs, in_maps)."""
    x = np.asarray(inputs["x"])
    Wr = np.asarray(inputs["Wr"])
    Wg = np.asarray(inputs["Wg"])
    Wu = np.asarray(inputs["Wu"])
    Wd = np.asarray(inputs["Wd"])
    xf = x.reshape(-1, D).astype(np.float32, copy=False)

    top2, s1, s2 = _route(xf, Wr)

    mmnp = np.dtype(np.float16 if MM_DTYPE == "float16" else np.float32)
    if MM_DTYPE == "bfloat16":
        import ml_dtypes

        mmnp = np.dtype(ml_dtypes.bfloat16)

    xf_mm = xf.astype(mmnp)

    idxs = []
    counts = []
    for e in range(E):
        idx = np.nonzero((top2[:, 0] == e) | (top2[:, 1] == e))[0]
        idxs.append(idx)
        counts.append(len(idx))
    cap = max(MIN_CAP, -(-max(counts) // 128) * 128)

    in_maps = []
    for e in range(E):
        idx = idxs[e]
        n_e = len(idx)
        xt = np.zeros((D, cap), dtype=mmnp)
        xt[:, :n_e] = xf_mm[idx].T
        sc = np.zeros(cap, dtype=np.float32)
        sc[:n_e] = np.where(top2[idx, 0] == e, s1[idx], s2[idx])
        sc2d = np.ascontiguousarray(sc.reshape(cap // 128, 128).T)
        in_maps.append(
            {
                "xt": xt.reshape(ND, 128, cap),
                "wg": np.ascontiguousarray(Wg[e].T).astype(mmnp).reshape(ND, 128, FF),
                "wu": np.ascontiguousarray(Wu[e].T).astype(mmnp).reshape(ND, 128, FF),
                "wd": np.ascontiguousarray(Wd[e].T).astype(mmnp).reshape(NF, 128, D),
                "sc": sc2d,
            }
        )
    return cap, idxs, in_maps


def kernel(**inputs):
    global LAST_RESULTS
    from concourse.bass_utils import run_bass_kernel_spmd

    x = np.asarray(inputs["x"])
    B, T, _ = x.shape
    n_tok = B * T

    cap, idxs, in_maps = prepare(inputs)
    nc = _get_nc(cap)
    res = run_bass_kernel_spmd(nc, in_maps, list(range(NCORES)))
    LAST_RESULTS = res

    out = np.zeros((n_tok, D), dtype=np.float32)
    for e in range(E):
        idx = idxs[e]
        out[idx] += res.results[e]["y"].reshape(-1, D)[: len(idx)]
    return out.reshape(B, T, D).astype(x.dtype, copy=False)



# revision 19
# speedup vs baseline: 5.0818x; 2.3256x over previous
"""FF-sharded Trainium2 Bass kernel for top-2 MoE (nn_MoE_2113123910117).

Strategy: shard the FF dimension (2048) of every expert across the 8
cores — core c owns f-channels [c*256, (c+1)*256) of ALL 8 experts.
Every core processes ALL token-expert assignments (grouped per expert,
each expert's token list padded to a multiple of 128), computing a
rank-256 partial of each expert FFN; the host sums the 8 partials.

vs. expert-parallel: no capacity padding to the max expert load — the
slot count is sum(pad128(load_e)) = ~16768 instead of 8*2176 = 17408,
and per-core work is identical by construction (no load imbalance).

All SBUF streams use packed 3D-AP DMAs (one descriptor per token tile /
weight round) — many small per-d-chunk DMAs serialize on the DGE queue.

Self-contained: hardcodes all shapes from the problem spec.
"""

import os
import numpy as np

D = 1024
FF = 2048
E = 8
TOPK = 2
NCORES = 8
ND = D // 128      # contraction chunks
FS = FF // NCORES  # per-core FF slice (256)
NFS = FS // 128    # f-tiles per expert per core (2)
TT = 512           # token tile

MM_DTYPE = os.environ.get("MOE_MM_DTYPE", "float16")

LAST_RESULTS = None
_NC_CACHE = {}


def split_multi_waits(nc, mybir_mod):
    """This walrus build rejects any instruction carrying more than one
    sync wait ("Too many sync wait commands"). Hoist extra waits onto
    single-wait NOPs inserted just before the instruction on the same
    engine — semantically identical since engines execute in order."""
    n_split = 0
    for f in nc.m.functions:
        for blk in f.blocks:
            insts = blk.instructions
            newl = []
            changed = False
            for inst in insts:
                si = inst.sync_info
                if si is not None and len(si.on_wait) > 1:
                    waits = list(si.on_wait)
                    del si.on_wait[1:]
                    for j, w in enumerate(waits[1:]):
                        nop = mybir_mod.InstNoOp(
                            name=f"{inst.name}_w{j}",
                            engine=inst.engine,
                            ins=[],
                            outs=[],
                        )
                        nop.sync_info = mybir_mod.SyncInfo(on_wait=[w], on_update=[])
                        newl.append(nop)
                        n_split += 1
                    changed = True
                newl.append(inst)
            if changed:
                insts[:] = newl
    return n_split


def _seg_tiles(seg_lens):
    """Global (expert, off, tt) token tiles, segment-contiguous."""
    tiles = []
    off = 0
    for e, L in enumerate(seg_lens):
        o = 0
        while o < L:
            tiles.append((e, off + o, min(TT, L - o)))
            o += TT
        off += L
    return tiles


def build_nc(seg_lens, repeat=1):
    """Per-core Bass program: for each expert segment, the rank-FS partial
    SwiGLU FFN over its (padded) tokens. Same NEFF on all 8 cores; the
    per-core FF slice lives entirely in the input weight layout."""
    import contextlib

    import concourse.bass as bass
    import concourse.mybir as mybir
    import concourse.tile as tile

    dt = mybir.dt
    f32 = dt.float32
    mmdt = getattr(dt, MM_DTYPE)
    AF = mybir.ActivationFunctionType
    NS = sum(seg_lens)
    NG = NS // 128

    nc = bass.Bass()
    # 3D dram layouts so a whole [128, chunks, cols] SBUF tile fills in
    # one descriptor (row-major identical to the natural 2D layouts)
    xt = nc.dram_tensor("xt", [ND, 128, NS], mmdt, kind="ExternalInput")
    wg = nc.dram_tensor("wg", [ND, 128, E * FS], mmdt, kind="ExternalInput")
    wu = nc.dram_tensor("wu", [ND, 128, E * FS], mmdt, kind="ExternalInput")
    wd = nc.dram_tensor("wd", [E * NFS, 128, D], mmdt, kind="ExternalInput")
    sc = nc.dram_tensor("sc", [128, NG], f32, kind="ExternalInput")
    y = nc.dram_tensor("y", [NG, 128, D], mmdt, kind="ExternalOutput")

    tiles = _seg_tiles(seg_lens)

    with tile.TileContext(nc) as tc:
        with (
            tc.tile_pool(name="wpool", bufs=1) as wpool,
            tc.tile_pool(name="xpool", bufs=3) as xpool,
            tc.tile_pool(name="hpool", bufs=2) as hpool,
            tc.tile_pool(name="gpool", bufs=3) as gpool,
            tc.tile_pool(name="ypool", bufs=2) as ypool,
            tc.tile_pool(name="pg", bufs=2, space="PSUM") as pgpool,
            tc.tile_pool(name="pu", bufs=2, space="PSUM") as pupool,
            tc.tile_pool(name="po", bufs=4, space="PSUM") as popool,
            (
                tc.For_i(0, repeat, 1, hint_engines=(mybir.EngineType.PE,))
                if repeat > 1
                else contextlib.nullcontext()
            ),
        ):
            # first x tile + scales ahead of the weight stream on SP
            e0, off0, tt0 = tiles[0]
            xt0 = xpool.tile([128, ND, tt0], mmdt, tag="xt")
            nc.sync.dma_start(
                xt0[:], xt[:, :, off0 : off0 + tt0].rearrange("d p t -> p d t")
            )
            s_sb = wpool.tile([128, NG], f32, tag="s")
            nc.sync.dma_start(s_sb[:], sc[:])

            # weights on SP in expert-pair rounds (first round = first
            # experts the token stream needs), one descriptor per stream
            wg_sb = wpool.tile([128, ND, E * FS], mmdt, tag="wg")
            wu_sb = wpool.tile([128, ND, E * FS], mmdt, tag="wu")
            wd_sb = wpool.tile([128, E * NFS, D], mmdt, tag="wd")
            for ep in range(0, E, 2):
                c0, c1 = ep * FS, (ep + 2) * FS
                nc.sync.dma_start(
                    wg_sb[:, :, c0:c1],
                    wg[:, :, c0:c1].rearrange("d p c -> p d c"),
                )
                nc.sync.dma_start(
                    wu_sb[:, :, c0:c1],
                    wu[:, :, c0:c1].rearrange("d p c -> p d c"),
                )
                i0, i1 = ep * NFS, (ep + 2) * NFS
                nc.sync.dma_start(
                    wd_sb[:, i0:i1, :],
                    wd[i0:i1, :, :].rearrange("i p c -> p i c"),
                )

            # main loop, down-projection software-pipelined by one tile:
            # PE order is [g/u t0][g/u t1][down t0][g/u t2][down t1]...
            # so down's ht operands always have a full tile of slack.
            pend = None

            def emit_down(e, off, tt, ht_t):
                nk = tt // 128
                yt = ypool.tile([128, nk, D], mmdt, tag="yt", name=f"yt_{off}")
                for k in range(nk):
                    g = off // 128 + k
                    po_h = []
                    for dh in range(2):
                        po = popool.tile(
                            [128, 512], f32, tag="po", name=f"po_{off}_{k}_{dh}"
                        )
                        po_h.append(po)
                    for fi in range(NFS):
                        lhs = ht_t[fi][:, k * 128 : (k + 1) * 128]
                        for dh in range(2):
                            nc.tensor.matmul(
                                po_h[dh][:],
                                lhs,
                                wd_sb[:, e * NFS + fi, dh * 512 : (dh + 1) * 512],
                                start=(fi == 0),
                                stop=(fi == NFS - 1),
                            )
                    # split the scaled PSUM->SBUF copies across Act and DVE:
                    # 8 copies/tile on Act alone can't keep up with PE's
                    # down phase and throttle the po PSUM ring
                    nc.scalar.activation(
                        yt[:, k, 0:512],
                        po_h[0][:],
                        AF.Copy,
                        scale=s_sb[:, g : g + 1],
                    )
                    nc.vector.tensor_scalar_mul(
                        yt[:, k, 512:1024], po_h[1][:], s_sb[:, g : g + 1]
                    )
                g0 = off // 128
                nc.scalar.dma_start(
                    y[g0 : g0 + nk, :, :].rearrange("g p c -> p g c"), yt[:]
                )

            for ti, (e, off, tt) in enumerate(tiles):
                if ti == 0:
                    xt_t = xt0
                else:
                    xt_t = xpool.tile([128, ND, tt], mmdt, tag="xt")
                    nc.scalar.dma_start(
                        xt_t[:],
                        xt[:, :, off : off + tt].rearrange("d p t -> p d t"),
                    )
                ht_t = []
                for fi in range(NFS):
                    fc = e * FS + fi * 128
                    pg = pgpool.tile([128, tt], f32, tag="pg")
                    pu = pupool.tile([128, tt], f32, tag="pu")
                    for d in range(ND):
                        nc.tensor.matmul(
                            pg[:],
                            wg_sb[:, d, fc : fc + 128],
                            xt_t[:, d, :],
                            start=(d == 0),
                            stop=(d == ND - 1),
                        )
                    for d in range(ND):
                        nc.tensor.matmul(
                            pu[:],
                            wu_sb[:, d, fc : fc + 128],
                            xt_t[:, d, :],
                            start=(d == 0),
                            stop=(d == ND - 1),
                        )
                    sg = gpool.tile([128, tt], mmdt, tag="sg")
                    nc.scalar.activation(sg[:], pg[:], AF.Silu)
                    ht = hpool.tile([128, tt], mmdt, tag=f"ht{fi}")
                    nc.vector.tensor_mul(ht[:], sg[:], pu[:])
                    ht_t.append(ht)
                if pend is not None:
                    emit_down(*pend)
                pend = (e, off, tt, ht_t)
            emit_down(*pend)

    split_multi_waits(nc, mybir)
    return nc


def _get_nc(seg_lens):
    key = (tuple(seg_lens), MM_DTYPE)
    if key not in _NC_CACHE:
        _NC_CACHE[key] = build_nc(tuple(seg_lens))
    return _NC_CACHE[key]


def _route(xf, Wr):
    """fp32 softmax + top-2 + normalized combine weights, matching the
    jax reference (ties broken toward lower expert index)."""
    logits = xf @ Wr.astype(np.float32).T
    m = logits.max(-1, keepdims=True)
    ex = np.exp(logits - m)
    p = ex / ex.sum(-1, keepdims=True)
    top2 = np.argsort(-p, axis=-1, kind="stable")[:, :TOPK]
    n = xf.shape[0]
    p1 = p[np.arange(n), top2[:, 0]]
    p2 = p[np.arange(n), top2[:, 1]]
    denom = (p1 + p2) + np.float32(1e-8)
    return top2, p1 / denom, p2 / denom


def prepare(inputs):
    """Host dispatch: returns (seg_lens, idxs, in_maps)."""
    x = np.asarray(inputs["x"])
    Wr = np.asarray(inputs["Wr"])
    Wg = np.asarray(inputs["Wg"])
    Wu = np.asarray(inputs["Wu"])
    Wd = np.asarray(inputs["Wd"])
    xf = x.reshape(-1, D).astype(np.float32, copy=False)

    top2, s1, s2 = _route(xf, Wr)

    mmnp = np.dtype(np.float16 if MM_DTYPE == "float16" else np.float32)
    if MM_DTYPE == "bfloat16":
        import ml_dtypes

        mmnp = np.dtype(ml_dtypes.bfloat16)
    xf_mm = xf.astype(mmnp)

    idxs = []
    seg_lens = []
    for e in range(E):
        idx = np.nonzero((top2[:, 0] == e) | (top2[:, 1] == e))[0]
        idxs.append(idx)
        seg_lens.append(-(-len(idx) // 128) * 128)
    NS = sum(seg_lens)

    xt_all = np.zeros((D, NS), dtype=mmnp)
    sc = np.zeros(NS, dtype=np.float32)
    off = 0
    for e in range(E):
        idx = idxs[e]
        n_e = len(idx)
        xt_all[:, off : off + n_e] = xf_mm[idx].T
        sc[off : off + n_e] = np.where(top2[idx, 0] == e, s1[idx], s2[idx])
        off += seg_lens[e]
    sc2d = np.ascontiguousarray(sc.reshape(NS // 128, 128).T)
    xt_all = xt_all.reshape(ND, 128, NS)

    in_maps = []
    for c in range(NCORES):
        sl = slice(c * FS, (c + 1) * FS)
        wg_c = np.concatenate(
            [np.ascontiguousarray(Wg[e].T[:, sl]) for e in range(E)], axis=1
        ).astype(mmnp).reshape(ND, 128, E * FS)
        wu_c = np.concatenate(
            [np.ascontiguousarray(Wu[e].T[:, sl]) for e in range(E)], axis=1
        ).astype(mmnp).reshape(ND, 128, E * FS)
        wd_c = np.concatenate(
            [np.ascontiguousarray(Wd[e].T[sl, :]) for e in range(E)], axis=0
        ).astype(mmnp).reshape(E * NFS, 128, D)
        in_maps.append(
            {"xt": xt_all, "wg": wg_c, "wu": wu_c, "wd": wd_c, "sc": sc2d}
        )
    return seg_lens, idxs, in_maps


def kernel(**inputs):
    global LAST_RESULTS
    from concourse.bass_utils import run_bass_kernel_spmd

    x = np.asarray(inputs["x"])
    B, T, _ = x.shape
    n_tok = B * T

    seg_lens, idxs, in_maps = prepare(inputs)
    nc = _get_nc(seg_lens)
    res = run_bass_kernel_spmd(nc, in_maps, list(range(NCORES)))
    LAST_RESULTS = res

    acc = res.results[0]["y"].astype(np.float32)
    for c in range(1, NCORES):
        acc += res.results[c]["y"]
    acc = acc.reshape(-1, D)

    out = np.zeros((n_tok, D), dtype=np.float32)
    off = 0
    for e in range(E):
        idx = idxs[e]
        out[idx] += acc[off : off + len(idx)]
        off += seg_lens[e]
    return out.reshape(B, T, D).astype(x.dtype, copy=False)
